# revision 28
# baseline (speedup 1.0000x reference)
"""GRU-style segmented-scan aggregator for Trainium2 (8 NeuronCores).

Reference computes, per node n with messages m_1..m_L sorted by time t:
    h <- W @ (m + h) + b   starting from h = 0
and returns the final h per node (zeros for empty nodes).

Because every step uses the SAME matrix W, the final state has the closed
form (h_0 = 0):
    h = sum_{k=0}^{L-1} W^{k+1} m_{(L-1-k)}  +  S_L b,   S_L = sum_{p<L} W^p
i.e. the k-th message FROM THE END is hit by W^{k+1}.  This turns the
sequential scan into independent batched matmuls against precomputed powers
of W -- ideal for the PE array.

Device layout (per core, SPMD over 8 cores), all data bf16 (PSUM fp32):
  - nodes are sorted by message count (desc) and dealt round-robin to cores;
    each core owns <=1024 node slots, columns of a [256 feat x 1024] H^T
    accumulator kept in PSUM (4 tiles: 2 feature chunks x 2 slot banks).
  - step k < K0 multiplies W^{k+1} (lhsT, 4 chunks of 128x128) with the
    k-th-from-end messages of the first n_k slots (rhs, features on
    partitions), and accumulates into PSUM.  n_k shrinks as shorter segments
    are exhausted, so there is ~0% padding.
  - deep steps K0 <= k < K0+TAILK reuse the SAME weight tiles via the
    commutation  W^{K0+j+1} m = W^{j+1} (W^{K0} m) : the host pre-multiplies
    the (few) tail messages by W^{K0} in fp64, so the device accumulates
    them straight into the main PSUM with weights W^1..W^TAILK already on
    chip.  Only the (tiny) remainder k >= K0+TAILK is folded into the
    per-node bias term on the host.
  - the bias term S_L b (+ host-folded remainder) is pre-multiplied by
    W^{-1} on the host (fp64) and added to each slot's k=0 message, so no
    device instructions are spent on it at all.
  - writeback is staged per PSUM bank: once no further step touches a bank
    it is copied out (scalar engine for feature chunk 0, vector engine for
    chunk 1, casting to bf16) and DMA'd while the PE keeps working.
  - matmuls on a zeroed scratch tile warm the PE p-state while the first
    DMA block is in flight; step 0 is split into four sub-DMAs so real
    matmuls can start as early as possible.

Host does the (cheap) data marshalling: lexsort by (index, t), gather into
the k-major column layout, precompute W powers in fp64, scatter results back.
"""

import numpy as np

import concourse.bass as bass
import concourse.mybir as mybir
from concourse import tile
from concourse.bass_utils import run_bass_kernel_spmd
import bass_rust

_N_PROCS = 27


class _SplitDrainTC(tile.TileContext):
    """TileContext whose kernel-tail drain is split into one drain per proc.

    The walrus build in this container rejects instructions carrying more
    than one sync wait; the stock tail drain waits on every proc at once.
    Emitting a chain of drains, each waiting on a single semaphore, is
    semantically identical (all procs quiesced before the exit barrier).
    """

    def _drain_and_barrier(self, tick_clock, wait_clock):
        gc = tick_clock.global_clock
        for p in range(_N_PROCS):
            if gc[p] <= 0:
                continue
            d = self.nc.sync.drain()
            vc = bass_rust.VectorClock(
                [gc[q] if q == p else 0 for q in range(_N_PROCS)])
            wait_clock.add_sem_waits(d.ins, bass_rust.ScopedClock({None: vc}))
        assert self.sems is not None
        popped = self.nc._tile_sem_poison_stack.pop()
        assert popped is self._sem_poison
        self.nc.all_engine_barrier()
        self.nc.clear_and_free_semaphores(list(self.sems.allocated().values()))
        self.nc.all_engine_barrier()

N_CORES = 8
DIM = 256
SLOTS = 1024   # node slots per core == PSUM accumulator width
K0_MAX = 18    # weight powers shipped to the device
TAILK_MAX = 6  # deep steps handled on device by weight reuse + one apply
N_WARM = 9     # PE p-state warmup matmuls (512 cols each)

_BF16 = None   # set lazily (mybir.dt.np)

_NC_CACHE: dict = {}


def _build_nc(K0: int, TAILK: int, n_k: tuple, n_t: tuple, stages: tuple,
              flo: int, Q: int):
    """Build the Bass program for one core (shared by all 8 via SPMD).

    This walrus build accepts at most ONE sync wait per instruction, so the
    kernel is written with zero SBUF-slot reuse (every stream block gets its
    own tile), weight + message columns of each step share one DMA so every
    matmul has a single producer, and each PSUM tile is read exactly once.
    """
    f32 = mybir.dt.float32
    bf16 = mybir.dt.bfloat16
    nc = bass.Bass()

    nts = n_t[0] if TAILK else 0
    Ct = sum(n_t)

    mw = nc.dram_tensor("mw", [128, Q], bf16, kind="ExternalInput")
    out = nc.dram_tensor("out", [128, 2 * SLOTS], bf16, kind="ExternalOutput")

    with _SplitDrainTC(nc) as tc:
        with (
            tc.tile_pool(name="m", bufs=1) as mpool,
            tc.tile_pool(name="misc", bufs=1) as miscpool,
            tc.tile_pool(name="ps", bufs=1, space="PSUM") as pspool,
        ):
            # one PSUM tile per (feature chunk, slot bank): each tile is
            # read by exactly one copy engine exactly once, so no
            # instruction ever needs a second (cross-tile WAR) wait
            phs = [[pspool.tile([128, 512], f32, tag=f"ph{i}_{sb}",
                                name=f"ph{i}_{sb}")
                    for sb in range(SLOTS // 512)] for i in range(2)]
            pwarm = pspool.tile([128, 512], f32, tag="pw", name="pw")

            # -- PE warmup on a zeroed tile: spans the ~4.5us first-DMA
            # completion latency so the clock-ramp window elapses and the
            # activity monitor never sees an idle gap --
            wtile = miscpool.tile([128, 512], bf16, tag="wu", name="wu")
            nc.gpsimd.memset(wtile[:], 0)   # pool engine frees up first
            for _ in range(N_WARM):
                nc.tensor.matmul(pwarm[:], wtile[:, 0:128], wtile[:],
                                 start=True, stop=True, skip_group_check=True)

            # -- input streams: [512 wt | n_k hi | n_k lo] per step, tail
            #    messages ride last --
            assert n_k[0] == SLOTS
            mks = []
            q = 0
            for k in range(K0):
                blk = 512 + 2 * n_k[k]
                mk = mpool.tile([128, blk], bf16, tag=f"mk{k}", name=f"mk{k}")
                nc.sync.dma_start(mk[:], mw[:, q:q + blk])
                q += blk
                mks.append(mk)
            tb = None
            if TAILK:
                tb = mpool.tile([128, 2 * Ct], bf16, tag="tb", name="tb")
                nc.sync.dma_start(tb[:], mw[:, q:q + 2 * Ct])
                q += 2 * Ct
            assert q == Q

            def wslice(k, i, j):
                return mks[k][:, j * 256 + i * 128: j * 256 + (i + 1) * 128]

            def rslice(k, j, s, e):
                nk = n_k[k]
                return mks[k][:, 512 + j * nk + s: 512 + j * nk + e]

            def writeback(lo: int, hi: int, final: bool = False):
                # copy the closed bank out on two engines, then store
                sb = lo // 512
                assert hi - lo <= 512 and sb == (hi - 1) // 512
                obs = []
                for i in range(2):
                    ob = miscpool.tile([128, hi - lo], bf16,
                                       tag=f"ob{i}_{lo}", name=f"ob{i}_{lo}")
                    if i == 0:
                        nc.scalar.copy(ob[:], phs[i][sb][:, lo - sb * 512:
                                                         hi - sb * 512])
                    else:
                        nc.vector.tensor_copy(ob[:], phs[i][sb][:, lo - sb * 512:
                                                                hi - sb * 512])
                    obs.append(ob)
                # SWDGE: fresh DMASW lanes -> single producer wait (a sync
                # HWDGE store would reuse a completion lane = 2nd wait)
                for i in range(2):
                    nc.gpsimd.dma_start(out[:, i * SLOTS + lo: i * SLOTS + hi],
                                        obs[i][:])

            # -- main steps --
            # last step that touches each slot bank (stop flag bookkeeping)
            last_k = [max(k for k in range(K0) if n_k[k] > sb * 512)
                      for sb in range(SLOTS // 512)]
            for k in range(K0):
                nk = n_k[k]
                for i in range(2):          # output feature chunk
                    for j in range(2):      # contraction chunk
                        for s in range(0, nk, 512):
                            e = min(nk, s + 512)
                            sb = s // 512
                            fin = (k == last_k[sb] and j == 1
                                   and not (sb == 0 and TAILK))
                            nc.tensor.matmul(
                                phs[i][sb][:, 0:e - s], wslice(k, i, j),
                                rslice(k, j, s, e),
                                start=(k == 0 and j == 0), stop=fin,
                                skip_group_check=True)
                for trig, lo, hi in stages:
                    if trig == k + 1:
                        writeback(lo, hi)

            # -- tail steps: host pre-multiplied the tail messages by W^K0,
            #    so they accumulate straight into the main PSUM with the
            #    weight tiles already on chip --
            if TAILK:
                off = 0
                for jl in range(TAILK):
                    ntj = n_t[jl]
                    for i in range(2):
                        for jj in range(2):
                            nc.tensor.matmul(
                                phs[i][0][:, 0:ntj], wslice(jl, i, jj),
                                tb[:, off + jj * ntj: off + (jj + 1) * ntj],
                                start=False,
                                stop=(jl == TAILK - 1 and jj == 1),
                                skip_group_check=True)
                    off += 2 * ntj

            # -- final region(s) --
            for lo in range(0, flo, 512):
                writeback(lo, min(flo, lo + 512), final=True)
    return nc


def _prepare(msg, index, t, dim_size, W, b):
    """Host-side marshalling. Returns (in_maps, node_ids, schedule key)."""
    global _BF16
    if _BF16 is None:
        _BF16 = mybir.dt.np(mybir.dt.bfloat16)
    E, D = msg.shape
    counts = np.bincount(index, minlength=dim_size)
    order = np.lexsort((t, index))            # stable: primary index, secondary t
    msg_sorted = msg[order]                   # [E, D] grouped by node, t-ascending
    seg_starts = np.zeros(dim_size, np.int64)
    seg_starts[1:] = np.cumsum(counts)[:-1]

    nodesort = np.argsort(-counts, kind="stable")
    nz = nodesort[counts[nodesort] > 0]
    per_core = -(-len(nz) // N_CORES)
    assert per_core <= SLOTS, f"too many nodes per core: {per_core}"

    node_ids = np.full((N_CORES, SLOTS), -1, np.int64)
    for c in range(N_CORES):
        ids = nz[c::N_CORES]
        node_ids[c, :len(ids)] = ids
    cc = np.where(node_ids >= 0, counts[np.maximum(node_ids, 0)], 0)  # [8, SLOTS]

    Lmax = int(cc.max())
    n_all = [int((cc > k).sum(axis=1).max()) for k in range(Lmax)]
    if n_all:
        n_all[0] = SLOTS          # every PSUM column is start=True'd at k=0

    K0 = min(K0_MAX, Lmax)
    TAILK = min(TAILK_MAX, max(0, Lmax - K0), K0)
    if TAILK > 0 and n_all[K0] > 512:
        TAILK = 0                 # tail accumulator must fit one PSUM bank
    n_k = tuple(n_all[:K0])
    n_t = tuple(n_all[K0:K0 + TAILK])
    nts = n_t[0] if TAILK else 0
    Cdev = int(sum(n_k))
    Ct = int(sum(n_t))

    # staged writeback: [lo,hi) closes after the last step that touches it.
    # stage boundaries MUST be PSUM-bank aligned (512 fp32): the copy engines
    # read the closed bank while the PE still writes the lower bank, and
    # PE-write + DVE/ACT-read of the SAME bank is fatal on TRN2.
    stages = []
    flo = SLOTS
    for lo, hi in ((512, 1024),):
        trig = next((k for k in range(K0) if n_k[k] <= lo), K0)
        if 0 < trig <= K0 and lo >= nts and hi == flo:
            stages.append((trig, lo, hi))
            flo = lo
    stages = tuple(stages)
    assert nts <= flo

    # column -> position in msg_sorted (or -1 = zero pad); main k-major,
    # then tail levels
    rowidx = np.full((N_CORES, Cdev + Ct), -1, np.int64)
    off = 0
    for k in list(range(K0)) + list(range(K0, K0 + TAILK)):
        nk = n_all[k] if k < len(n_all) else 0
        nid = node_ids[:, :nk]
        ck = cc[:, :nk]
        active = k < ck
        pos = seg_starts[np.maximum(nid, 0)] + ck - 1 - k
        rowidx[:, off:off + nk] = np.where(active, pos, -1)
        off += nk

    # weights: powers of W in fp64, stored transposed (lhsT chunks).
    Wd = W.astype(np.float64)
    bd = b.astype(np.float64)
    wfull = np.empty((128, K0 * 512), np.float32)
    s_table = np.zeros((Lmax + 1, D), np.float64)   # s_p = S_p b
    Wpows = []                                      # W^{k+1} (fp64)
    P = Wd.copy()
    for k in range(Lmax):
        if k < K0:
            WT = P.T.astype(np.float32)             # (W^{k+1}).T
            wfull[:, k * 512:k * 512 + 256] = WT[:128, :]
            wfull[:, k * 512 + 256:(k + 1) * 512] = WT[128:, :]
        Wpows.append(P)
        s_table[k + 1] = Wd @ s_table[k] + bd
        P = P @ Wd

    # per-(core, slot) bias term: S_L b plus host-folded remainder (k beyond
    # the device tail; a fraction of a percent of all messages).  The whole
    # term is pre-multiplied by W^{-1} and added to the k=0 message, whose
    # W^1 matmul then reproduces it -- zero device cost.
    bterm = s_table[cc]                              # [8, SLOTS, 256] fp64
    for k in range(K0 + TAILK, Lmax):
        nk = n_all[k]
        act = k < cc[:, :nk]                         # [8, nk]
        cs, ss = np.nonzero(act)
        pos = seg_starts[node_ids[cs, ss]] + cc[cs, ss] - 1 - k
        Y = msg_sorted[pos].astype(np.float64) @ Wpows[k].T
        bterm[cs, ss] += Y
    xfold = np.linalg.solve(Wd, bterm.reshape(-1, D).T).T.reshape(bterm.shape)
    xfold32 = xfold.astype(np.float32)
    PK0T32 = Wpows[K0 - 1].T.astype(np.float32) if TAILK else None  # (W^K0).T

    Q = K0 * 512 + 2 * Cdev + 2 * Ct
    in_maps = []
    for c in range(N_CORES):
        ri = rowidx[c]
        Mg = msg_sorted[np.maximum(ri, 0)]
        Mg[ri < 0] = 0.0                             # [Cdev+Ct, 256]
        Mg[:SLOTS] += xfold32[c]                     # bias fold into k=0
        if TAILK:                                    # tail rides W^K0 * m
            Mg[Cdev:] = Mg[Cdev:] @ PK0T32
        hi = Mg[:, :128].T                           # [128, Cdev+Ct]
        lo = Mg[:, 128:].T
        mwb = np.empty((128, Q), np.float32)
        off = 0
        q = 0
        for k in range(K0):
            nk = n_k[k]
            mwb[:, q:q + 512] = wfull[:, k * 512:(k + 1) * 512]
            mwb[:, q + 512:q + 512 + nk] = hi[:, off:off + nk]
            mwb[:, q + 512 + nk:q + 512 + 2 * nk] = lo[:, off:off + nk]
            off += nk
            q += 512 + 2 * nk
        for j in range(TAILK):
            ntj = n_t[j]
            mwb[:, q:q + ntj] = hi[:, off:off + ntj]
            mwb[:, q + ntj:q + 2 * ntj] = lo[:, off:off + ntj]
            off += ntj
            q += 2 * ntj
        assert q == Q
        in_maps.append({"mw": mwb.astype(_BF16)})
    return in_maps, node_ids, (K0, TAILK, n_k, n_t, stages, flo, Q)


def _run(inputs: dict, trace: bool = False, **run_kwargs):
    msg = np.ascontiguousarray(np.asarray(inputs["msg"], dtype=np.float32))
    index = np.asarray(inputs["index"]).astype(np.int64)
    t = np.asarray(inputs["t"], dtype=np.float32)
    W = np.asarray(inputs["W"], dtype=np.float32)
    b = np.asarray(inputs["b"], dtype=np.float32)
    dim_size = int(inputs["dim_size"])

    in_maps, node_ids, key = _prepare(msg, index, t, dim_size, W, b)
    if key not in _NC_CACHE:
        _NC_CACHE[key] = _build_nc(*key)
    nc = _NC_CACHE[key]

    res = run_bass_kernel_spmd(nc, in_maps, list(range(N_CORES)),
                               trace=trace, **run_kwargs)

    hidden = np.zeros((dim_size, DIM), np.float32)
    for c in range(N_CORES):
        o = np.asarray(res.results[c]["out"]).astype(np.float32)
        hc = np.concatenate([o[:, :SLOTS], o[:, SLOTS:]], axis=0).T  # [SLOTS, 256]
        valid = node_ids[c] >= 0
        hidden[node_ids[c][valid]] = hc[valid]
    return hidden, res


def kernel(**inputs) -> np.ndarray:
    hidden, _ = _run(inputs, trace=False)
    return hidden


# revision 29
# speedup vs baseline: 1.1937x; 1.1937x over previous
"""GRU-style segmented-scan aggregator for Trainium2 (8 NeuronCores).

Reference computes, per node n with messages m_1..m_L sorted by time t:
    h <- W @ (m + h) + b   starting from h = 0
and returns the final h per node (zeros for empty nodes).

Because every step uses the SAME matrix W, the final state has the closed
form (h_0 = 0):
    h = sum_{k=0}^{L-1} W^{k+1} m_{(L-1-k)}  +  S_L b,   S_L = sum_{p<L} W^p
i.e. the k-th message FROM THE END is hit by W^{k+1}.  This turns the
sequential scan into independent batched matmuls against precomputed powers
of W -- ideal for the PE array.

Device layout (per core, SPMD over 8 cores), all data bf16 (PSUM fp32):
  - nodes are sorted by message count (desc) and dealt round-robin to cores;
    each core owns <=1024 node slots, columns of a [256 feat x 1024] H^T
    accumulator kept in PSUM (4 tiles: 2 feature chunks x 2 slot banks).
  - step k < K0 multiplies W^{k+1} (lhsT, 4 chunks of 128x128) with the
    k-th-from-end messages of the first n_k slots (rhs, features on
    partitions), and accumulates into PSUM.  n_k shrinks as shorter segments
    are exhausted, so there is ~0% padding.
  - deep steps K0 <= k < K0+TAILK reuse the SAME weight tiles via the
    commutation  W^{K0+j+1} m = W^{j+1} (W^{K0} m) : the host pre-multiplies
    the (few) tail messages by W^{K0} in fp64, so the device accumulates
    them straight into the main PSUM with weights W^1..W^TAILK already on
    chip.  Only the (tiny) remainder k >= K0+TAILK is folded into the
    per-node bias term on the host.
  - the bias term S_L b (+ host-folded remainder) is pre-multiplied by
    W^{-1} on the host (fp64) and added to each slot's k=0 message, so no
    device instructions are spent on it at all.
  - writeback is staged per PSUM bank: once no further step touches a bank
    it is copied out (scalar engine for feature chunk 0, vector engine for
    chunk 1, casting to bf16) and DMA'd while the PE keeps working.
  - matmuls on a zeroed scratch tile warm the PE p-state while the first
    DMA block is in flight; step 0 is split into four sub-DMAs so real
    matmuls can start as early as possible.

Host does the (cheap) data marshalling: lexsort by (index, t), gather into
the k-major column layout, precompute W powers in fp64, scatter results back.
"""

import numpy as np

import concourse.bass as bass
import concourse.mybir as mybir
from concourse import tile
from concourse.bass_utils import run_bass_kernel_spmd
import bass_rust

_N_PROCS = 27


class _SplitDrainTC(tile.TileContext):
    """TileContext whose kernel-tail drain is split into one drain per proc.

    The walrus build in this container rejects instructions carrying more
    than one sync wait; the stock tail drain waits on every proc at once.
    Emitting a chain of drains, each waiting on a single semaphore, is
    semantically identical (all procs quiesced before the exit barrier).
    """

    def _drain_and_barrier(self, tick_clock, wait_clock):
        gc = tick_clock.global_clock
        for p in range(_N_PROCS):
            if gc[p] <= 0:
                continue
            d = self.nc.sync.drain()
            vc = bass_rust.VectorClock(
                [gc[q] if q == p else 0 for q in range(_N_PROCS)])
            wait_clock.add_sem_waits(d.ins, bass_rust.ScopedClock({None: vc}))
        assert self.sems is not None
        popped = self.nc._tile_sem_poison_stack.pop()
        assert popped is self._sem_poison
        self.nc.all_engine_barrier()
        self.nc.clear_and_free_semaphores(list(self.sems.allocated().values()))
        self.nc.all_engine_barrier()

N_CORES = 8
DIM = 256
SLOTS = 1024   # node slots per core == PSUM accumulator width
K0_MAX = 18    # weight powers shipped to the device
TAILK_MAX = 6  # deep steps handled on device by weight reuse + one apply
N_WARM = 9     # PE p-state warmup matmuls (512 cols each)

_BF16 = None   # set lazily (mybir.dt.np)

_NC_CACHE: dict = {}


def _build_nc(K0: int, TAILK: int, n_k: tuple, n_t: tuple, stages: tuple,
              flo: int, Q: int):
    """Build the Bass program for one core (shared by all 8 via SPMD).

    This walrus build accepts at most ONE sync wait per instruction, so the
    kernel is written with zero SBUF-slot reuse (every stream block gets its
    own tile), weight + message columns of each step share one DMA so every
    matmul has a single producer, and each PSUM tile is read exactly once.
    """
    f32 = mybir.dt.float32
    bf16 = mybir.dt.bfloat16
    nc = bass.Bass()

    nts = n_t[0] if TAILK else 0
    Ct = sum(n_t)

    mw = nc.dram_tensor("mw", [128, Q], bf16, kind="ExternalInput")
    out = nc.dram_tensor("out", [128, 2 * SLOTS], bf16, kind="ExternalOutput")

    with _SplitDrainTC(nc) as tc:
        with (
            tc.tile_pool(name="m", bufs=1) as mpool,
            tc.tile_pool(name="misc", bufs=1) as miscpool,
            tc.tile_pool(name="ps", bufs=1, space="PSUM") as pspool,
        ):
            # one PSUM tile per (feature chunk, slot bank): each tile is
            # read by exactly one copy engine exactly once, so no
            # instruction ever needs a second (cross-tile WAR) wait
            phs = [[pspool.tile([128, 512], f32, tag=f"ph{i}_{sb}",
                                name=f"ph{i}_{sb}")
                    for sb in range(SLOTS // 512)] for i in range(2)]
            pwarm = pspool.tile([128, 512], f32, tag="pw", name="pw")

            # -- PE warmup on a zeroed tile: spans the ~4.5us first-DMA
            # completion latency so the clock-ramp window elapses and the
            # activity monitor never sees an idle gap --
            wtile = miscpool.tile([128, 512], bf16, tag="wu", name="wu")
            nc.vector.memset(wtile[:], 0)
            for _ in range(N_WARM):
                nc.tensor.matmul(pwarm[:], wtile[:, 0:128], wtile[:],
                                 start=True, stop=True, skip_group_check=True)

            # -- input streams: [512 wt | n_k hi | n_k lo] per step, tail
            #    messages ride last --
            assert n_k[0] == SLOTS
            mks = []
            q = 0
            for k in range(K0):
                blk = 512 + 2 * n_k[k]
                mk = mpool.tile([128, blk], bf16, tag=f"mk{k}", name=f"mk{k}")
                nc.sync.dma_start(mk[:], mw[:, q:q + blk])
                q += blk
                mks.append(mk)
            tb = None
            if TAILK:
                tb = mpool.tile([128, 2 * Ct], bf16, tag="tb", name="tb")
                nc.sync.dma_start(tb[:], mw[:, q:q + 2 * Ct])
                q += 2 * Ct
            assert q == Q

            def wslice(k, i, j):
                return mks[k][:, j * 256 + i * 128: j * 256 + (i + 1) * 128]

            def rslice(k, j, s, e):
                nk = n_k[k]
                return mks[k][:, 512 + j * nk + s: 512 + j * nk + e]

            def writeback(lo: int, hi: int, final: bool = False):
                # copy the closed bank out on two engines, then store
                sb = lo // 512
                assert hi - lo <= 512 and sb == (hi - 1) // 512
                obs = []
                for i in range(2):
                    ob = miscpool.tile([128, hi - lo], bf16,
                                       tag=f"ob{i}_{lo}", name=f"ob{i}_{lo}")
                    if i == 0:
                        nc.scalar.copy(ob[:], phs[i][sb][:, lo - sb * 512:
                                                         hi - sb * 512])
                    else:
                        nc.vector.tensor_copy(ob[:], phs[i][sb][:, lo - sb * 512:
                                                                hi - sb * 512])
                    obs.append(ob)
                # SWDGE: fresh DMASW lanes -> single producer wait (a sync
                # HWDGE store would reuse a completion lane = 2nd wait)
                for i in range(2):
                    nc.gpsimd.dma_start(out[:, i * SLOTS + lo: i * SLOTS + hi],
                                        obs[i][:])

            # -- main steps --
            # last step that touches each slot bank (stop flag bookkeeping)
            last_k = [max(k for k in range(K0) if n_k[k] > sb * 512)
                      for sb in range(SLOTS // 512)]
            for k in range(K0):
                nk = n_k[k]
                for i in range(2):          # output feature chunk
                    for j in range(2):      # contraction chunk
                        for s in range(0, nk, 512):
                            e = min(nk, s + 512)
                            sb = s // 512
                            fin = (k == last_k[sb] and j == 1
                                   and not (sb == 0 and TAILK))
                            nc.tensor.matmul(
                                phs[i][sb][:, 0:e - s], wslice(k, i, j),
                                rslice(k, j, s, e),
                                start=(k == 0 and j == 0), stop=fin,
                                skip_group_check=True)
                for trig, lo, hi in stages:
                    if trig == k + 1:
                        writeback(lo, hi)

            # -- tail steps: host pre-multiplied the tail messages by W^K0,
            #    so they accumulate straight into the main PSUM with the
            #    weight tiles already on chip --
            if TAILK:
                off = 0
                for jl in range(TAILK):
                    ntj = n_t[jl]
                    for i in range(2):
                        for jj in range(2):
                            nc.tensor.matmul(
                                phs[i][0][:, 0:ntj], wslice(jl, i, jj),
                                tb[:, off + jj * ntj: off + (jj + 1) * ntj],
                                start=False,
                                stop=(jl == TAILK - 1 and jj == 1),
                                skip_group_check=True)
                    off += 2 * ntj

            # -- final region(s) --
            for lo in range(0, flo, 512):
                writeback(lo, min(flo, lo + 512), final=True)
    return nc


def _prepare(msg, index, t, dim_size, W, b):
    """Host-side marshalling. Returns (in_maps, node_ids, schedule key)."""
    global _BF16
    if _BF16 is None:
        _BF16 = mybir.dt.np(mybir.dt.bfloat16)
    E, D = msg.shape
    counts = np.bincount(index, minlength=dim_size)
    order = np.lexsort((t, index))            # stable: primary index, secondary t
    msg_sorted = msg[order]                   # [E, D] grouped by node, t-ascending
    seg_starts = np.zeros(dim_size, np.int64)
    seg_starts[1:] = np.cumsum(counts)[:-1]

    nodesort = np.argsort(-counts, kind="stable")
    nz = nodesort[counts[nodesort] > 0]
    per_core = -(-len(nz) // N_CORES)
    assert per_core <= SLOTS, f"too many nodes per core: {per_core}"

    node_ids = np.full((N_CORES, SLOTS), -1, np.int64)
    for c in range(N_CORES):
        ids = nz[c::N_CORES]
        node_ids[c, :len(ids)] = ids
    cc = np.where(node_ids >= 0, counts[np.maximum(node_ids, 0)], 0)  # [8, SLOTS]

    Lmax = int(cc.max())
    n_all = [int((cc > k).sum(axis=1).max()) for k in range(Lmax)]
    if n_all:
        n_all[0] = SLOTS          # every PSUM column is start=True'd at k=0

    K0 = min(K0_MAX, Lmax)
    TAILK = min(TAILK_MAX, max(0, Lmax - K0), K0)
    if TAILK > 0 and n_all[K0] > 512:
        TAILK = 0                 # tail accumulator must fit one PSUM bank
    n_k = tuple(n_all[:K0])
    n_t = tuple(n_all[K0:K0 + TAILK])
    nts = n_t[0] if TAILK else 0
    Cdev = int(sum(n_k))
    Ct = int(sum(n_t))

    # staged writeback: [lo,hi) closes after the last step that touches it.
    # stage boundaries MUST be PSUM-bank aligned (512 fp32): the copy engines
    # read the closed bank while the PE still writes the lower bank, and
    # PE-write + DVE/ACT-read of the SAME bank is fatal on TRN2.
    stages = []
    flo = SLOTS
    for lo, hi in ((512, 1024),):
        trig = next((k for k in range(K0) if n_k[k] <= lo), K0)
        if 0 < trig <= K0 and lo >= nts and hi == flo:
            stages.append((trig, lo, hi))
            flo = lo
    stages = tuple(stages)
    assert nts <= flo

    # column -> position in msg_sorted (or -1 = zero pad); main k-major,
    # then tail levels
    rowidx = np.full((N_CORES, Cdev + Ct), -1, np.int64)
    off = 0
    for k in list(range(K0)) + list(range(K0, K0 + TAILK)):
        nk = n_all[k] if k < len(n_all) else 0
        nid = node_ids[:, :nk]
        ck = cc[:, :nk]
        active = k < ck
        pos = seg_starts[np.maximum(nid, 0)] + ck - 1 - k
        rowidx[:, off:off + nk] = np.where(active, pos, -1)
        off += nk

    # weights: powers of W in fp64, stored transposed (lhsT chunks).
    Wd = W.astype(np.float64)
    bd = b.astype(np.float64)
    wfull = np.empty((128, K0 * 512), np.float32)
    s_table = np.zeros((Lmax + 1, D), np.float64)   # s_p = S_p b
    Wpows = []                                      # W^{k+1} (fp64)
    P = Wd.copy()
    for k in range(Lmax):
        if k < K0:
            WT = P.T.astype(np.float32)             # (W^{k+1}).T
            wfull[:, k * 512:k * 512 + 256] = WT[:128, :]
            wfull[:, k * 512 + 256:(k + 1) * 512] = WT[128:, :]
        Wpows.append(P)
        s_table[k + 1] = Wd @ s_table[k] + bd
        P = P @ Wd

    # per-(core, slot) bias term: S_L b plus host-folded remainder (k beyond
    # the device tail; a fraction of a percent of all messages).  The whole
    # term is pre-multiplied by W^{-1} and added to the k=0 message, whose
    # W^1 matmul then reproduces it -- zero device cost.
    bterm = s_table[cc]                              # [8, SLOTS, 256] fp64
    for k in range(K0 + TAILK, Lmax):
        nk = n_all[k]
        act = k < cc[:, :nk]                         # [8, nk]
        cs, ss = np.nonzero(act)
        pos = seg_starts[node_ids[cs, ss]] + cc[cs, ss] - 1 - k
        Y = msg_sorted[pos].astype(np.float64) @ Wpows[k].T
        bterm[cs, ss] += Y
    xfold = np.linalg.solve(Wd, bterm.reshape(-1, D).T).T.reshape(bterm.shape)
    xfold32 = xfold.astype(np.float32)
    PK0T32 = Wpows[K0 - 1].T.astype(np.float32) if TAILK else None  # (W^K0).T

    Q = K0 * 512 + 2 * Cdev + 2 * Ct
    in_maps = []
    for c in range(N_CORES):
        ri = rowidx[c]
        Mg = msg_sorted[np.maximum(ri, 0)]
        Mg[ri < 0] = 0.0                             # [Cdev+Ct, 256]
        Mg[:SLOTS] += xfold32[c]                     # bias fold into k=0
        if TAILK:                                    # tail rides W^K0 * m
            Mg[Cdev:] = Mg[Cdev:] @ PK0T32
        hi = Mg[:, :128].T                           # [128, Cdev+Ct]
        lo = Mg[:, 128:].T
        mwb = np.empty((128, Q), np.float32)
        off = 0
        q = 0
        for k in range(K0):
            nk = n_k[k]
            mwb[:, q:q + 512] = wfull[:, k * 512:(k + 1) * 512]
            mwb[:, q + 512:q + 512 + nk] = hi[:, off:off + nk]
            mwb[:, q + 512 + nk:q + 512 + 2 * nk] = lo[:, off:off + nk]
            off += nk
            q += 512 + 2 * nk
        for j in range(TAILK):
            ntj = n_t[j]
            mwb[:, q:q + ntj] = hi[:, off:off + ntj]
            mwb[:, q + ntj:q + 2 * ntj] = lo[:, off:off + ntj]
            off += ntj
            q += 2 * ntj
        assert q == Q
        in_maps.append({"mw": mwb.astype(_BF16)})
    return in_maps, node_ids, (K0, TAILK, n_k, n_t, stages, flo, Q)


def _run(inputs: dict, trace: bool = False, **run_kwargs):
    msg = np.ascontiguousarray(np.asarray(inputs["msg"], dtype=np.float32))
    index = np.asarray(inputs["index"]).astype(np.int64)
    t = np.asarray(inputs["t"], dtype=np.float32)
    W = np.asarray(inputs["W"], dtype=np.float32)
    b = np.asarray(inputs["b"], dtype=np.float32)
    dim_size = int(inputs["dim_size"])

    in_maps, node_ids, key = _prepare(msg, index, t, dim_size, W, b)
    if key not in _NC_CACHE:
        _NC_CACHE[key] = _build_nc(*key)
    nc = _NC_CACHE[key]

    res = run_bass_kernel_spmd(nc, in_maps, list(range(N_CORES)),
                               trace=trace, **run_kwargs)

    hidden = np.zeros((dim_size, DIM), np.float32)
    for c in range(N_CORES):
        o = np.asarray(res.results[c]["out"]).astype(np.float32)
        hc = np.concatenate([o[:, :SLOTS], o[:, SLOTS:]], axis=0).T  # [SLOTS, 256]
        valid = node_ids[c] >= 0
        hidden[node_ids[c][valid]] = hc[valid]
    return hidden, res


def kernel(**inputs) -> np.ndarray:
    hidden, _ = _run(inputs, trace=False)
    return hidden
